# revision 45
# baseline (speedup 1.0000x reference)
"""Trainium2 Bass kernel for CustomMultiHeadAttention.

Problem: B=2, S=2048, D=2048, H=16 heads of Hd=128; y = MHA(q, k, v) with
torch-Linear-style projections (x @ W.T + b) and out projection.

Sharding (8 NeuronCores): data-parallel over batch (2 groups) x tensor-
parallel over heads (4 heads = 512 channels per core). Each core computes
its 4 heads' Q/K/V projections, attention, and a column-sharded partial of
the out projection; the host sums the 4 partials per batch and adds
bo + Wo.T @ bv (the V bias contribution commutes through attention's
convex combination, so it is folded into a host-side constant).

Per-core device program (all matmuls bf16, K=128 contraction):
  - V projected sequence-major first (only needs xv + Wv), then K
    channels-major for all 4 heads, then per 1024-column s-half: Q proj,
    software-pipelined attention, and the partial out-projection.
  - attention inner loop is issue-order pipelined: scores for key-block
    i+2 are issued before P~V of key-block i, so the PE never waits on
    the ACT exp.
  - softmax denominator: DVE sums exp tiles into quad partials (bf16
    pair + quad adds), then a ones-column matmul per quad accumulates l
    in PSUM - a quarter of the PE stream cost of per-tile ones matmuls.
  - PE idle gaps are filled by interleaving: Q projection blocks for the
    next s-half ride inside attention heads of the current s-half, and
    out-projection blocks of the previous s-half ride inside attention
    heads of the next one.
  - normalization uses reciprocal_approx_fast (18-bit) + gpsimd
    partition broadcast, multiplied into O~ straight from PSUM.
"""

import math

import numpy as np
import ml_dtypes

B = 2
S = 2048
D = 2048
HD = 128          # head dim
TP = 4            # head-group (tensor-parallel) factor
CL = D // TP      # 512 local channels = 4 heads per core
NCORES = 8

_NC = None


def _build_nc(s=S, d=D, cl=CL):
    """Build the per-core Bass program (SPMD: same program, 8 cores)."""
    from contextlib import ExitStack

    import concourse.bass as bass
    import concourse.mybir as mybir
    import concourse.tile as tile
    from concourse import bacc, bass_isa

    f32 = mybir.dt.float32
    bf16 = mybir.dt.bfloat16
    Exp = mybir.ActivationFunctionType.Exp

    SBW = 512                 # single matmul max free dim / PSUM bank width
    W2 = 2 * SBW              # paired two-bank tile width
    NSB = s // SBW            # 4 s-blocks of 512
    NSP = s // W2             # 2 s-halves of 1024
    NTB = s // 128            # 16 key/value seq blocks
    KC = d // 128             # 16 contraction chunks over model dim
    NH = cl // HD             # 4 local heads
    SCALE = 1.0 / math.sqrt(HD)

    nc = bacc.Bacc("TRN2", target_bir_lowering=False, debug=False)

    # Inputs are host-pre-shuffled to the exact SBUF tile layouts so every
    # DMA line is 8KB contiguous (1KB lines cap a queue at ~100GB/s).
    # x*: [n*128+p, k, t] = x[n*512+t, k*128+p];  w*: [p, k, m] = W[m, k*128+p]
    NSB_ = s // 512
    xqT = nc.dram_tensor("xqT", [NSB_ * 128, d // 128, 512], bf16,
                         kind="ExternalInput").ap()
    xkT = nc.dram_tensor("xkT", [NSB_ * 128, d // 128, 512], bf16,
                         kind="ExternalInput").ap()
    xvT = nc.dram_tensor("xvT", [NSB_ * 128, d // 128, 512], bf16,
                         kind="ExternalInput").ap()
    wqT = nc.dram_tensor("wqT", [128, d // 128, cl], bf16,
                         kind="ExternalInput").ap()
    wkT = nc.dram_tensor("wkT", [128, d // 128, cl], bf16,
                         kind="ExternalInput").ap()
    wvT = nc.dram_tensor("wvT", [128, d // 128, cl], bf16,
                         kind="ExternalInput").ap()
    woT = nc.dram_tensor("woT", [128, cl // 128, d], bf16,
                         kind="ExternalInput").ap()
    bq = nc.dram_tensor("bq", [cl], f32, kind="ExternalInput").ap()
    bk = nc.dram_tensor("bk", [cl], f32, kind="ExternalInput").ap()
    zT = nc.dram_tensor("zT", [d, s], bf16, kind="ExternalOutput").ap()

    with tile.TileContext(nc) as tc, ExitStack() as ctx:
        const = ctx.enter_context(tc.tile_pool(name="const", bufs=1))
        wp = ctx.enter_context(tc.tile_pool(name="weights", bufs=4))
        kvp = ctx.enter_context(tc.tile_pool(name="kv", bufs=1))
        panels = ctx.enter_context(tc.tile_pool(name="panels", bufs=6))
        qtp = ctx.enter_context(tc.tile_pool(name="qt", bufs=1))
        otp = ctx.enter_context(tc.tile_pool(name="ot", bufs=2))
        p2p = ctx.enter_context(tc.tile_pool(name="ptiles", bufs=7))
        pap = ctx.enter_context(tc.tile_pool(name="padd", bufs=2))
        smalls = ctx.enter_context(tc.tile_pool(name="small", bufs=1))
        zpool = ctx.enter_context(tc.tile_pool(name="zout", bufs=3))
        # PSUM: rotating pair-bank pool (4) + PV accum (2) + l rows (1)
        ps_big = ctx.enter_context(tc.tile_pool(name="ps_big", bufs=2,
                                                space="PSUM"))
        ps_ops = ctx.enter_context(tc.tile_pool(name="ps_ops", bufs=1,
                                                space="PSUM"))
        ps_l = ctx.enter_context(tc.tile_pool(name="ps_l", bufs=1,
                                              space="PSUM"))
        ps_q = ctx.enter_context(tc.tile_pool(name="ps_q", bufs=1,
                                              space="PSUM"))

        ones_col = const.tile([128, 1], bf16, tag="ones_col")
        nc.vector.memset(ones_col, 1.0)
        ones_row_f32 = const.tile([1, 128], f32, tag="ones_row")
        nc.vector.memset(ones_row_f32, 1.0)
        warm = const.tile([128, SBW], bf16, tag="warm")
        nc.vector.memset(warm, 0.0)
        ps_warm = ps_q.tile([128, SBW], f32, tag="q", name="ps_warm")
        for _ in range(15):
            nc.tensor.matmul(ps_warm[0:1, :], lhsT=ones_col, rhs=warm,
                             start=True, stop=True, skip_group_check=True)

        KC2 = KC // 2
        KC4 = KC // 4

        # Weights and panels are split into k-halves (separate tiles) so
        # consumers can start as soon as the first half lands. DMAs are
        # spread across the two HWDGE queues (sync + scalar).
        def wpair(w_dram, nm, engs=None):
            pair = []
            for i in range(2):
                w = wp.tile([128, KC2, cl], bf16, tag="w", name=f"{nm}{i}")
                (engs[i] if engs else nc.sync).dma_start(
                    w, w_dram[:, i * KC2:(i + 1) * KC2])
                pair.append(w)
            return pair

        def wap(pair, k, m):
            return pair[k // KC2][:, k % KC2, m * 128:(m + 1) * 128]

        kt = [kvp.tile([128, s], bf16, tag=f"kt{h}", name=f"kt{h}")
              for h in range(NH)]
        vt = [kvp.tile([128, cl], bf16, tag=f"vt{t}", name=f"vt{t}")
              for t in range(NTB)]

        def xpanel(x_dram, n, nm, engs=None):
            src_ = x_dram[n * 128:(n + 1) * 128]
            xps = []
            for i in range(2):
                xp = panels.tile([128, KC2, SBW], bf16, tag="xpanel",
                                 name=f"{nm}_{i}")
                (engs[i] if engs else nc.sync).dma_start(
                    xp, src_[:, i * KC2:(i + 1) * KC2])
                xps.append(xp)
            return xps

        def xap(pair, k):
            return pair[k // KC2][:, k % KC2, :]

        # V path first: its weight + first panel gate the first matmul.
        # Both are loaded as interleaved k-QUARTER tiles, enqueued in
        # consumption order (the 16 shared DMA engines process packets
        # roughly in enqueue order, so order == priority): the first V
        # matmuls need only 0.75MB in flight.
        wv_sb = []
        xp0 = []
        for i in range(4):
            wq_t = wp.tile([128, KC4, cl], bf16, tag="wv", name=f"wv{i}",
                           bufs=4)
            nc.sync.dma_start(wq_t, wvT[:, i * KC4:(i + 1) * KC4])
            wv_sb.append(wq_t)
            xq_t = panels.tile([128, KC4, SBW], bf16, tag="xpanel",
                               name=f"xpv0_{i}")
            nc.sync.dma_start(xq_t, xvT[0:128, i * KC4:(i + 1) * KC4])
            xp0.append(xq_t)

        def wvap(k):
            return wv_sb[k // KC4][:, k % KC4, :]

        def xap0(k):
            return xp0[k // KC4][:, k % KC4, :]

        def load_bias(b_dram, nm):
            # SWDGE: keeps the 512 tiny 4B packets off the HW DMA engines
            b_dma = const.tile([128, NH], f32, tag=f"{nm}d", name=f"{nm}d")
            nc.gpsimd.dma_start(b_dma,
                                b_dram.rearrange("(m p) -> p m", p=128))
            b_sb = const.tile([128, NH], f32, tag=nm, name=nm)
            nc.vector.tensor_copy(b_sb, b_dma)
            return b_sb

        bq_sb = load_bias(bq, "bq")
        bk_sb = load_bias(bk, "bk")

        # ---- V projection: vt[t][tt, e] = sum_d x[t*128+tt, d] Wv[e, d]
        # Panel 0 runs k-quarter-major (both psum pairs live) so its first
        # matmuls need only the first quarters of wv + xpv0. The last
        # quarter goes pair-major so pair-0 drains overlap pair-1 matmuls.
        ps_pair = [ps_big.tile([128, W2], f32, tag="mm", name=f"ps_v0{p}")
                   for p in range(2)]
        for kq in range(3):
            for k in range(kq * KC4, (kq + 1) * KC4):
                for pair in range(2):
                    for half in range(2):
                        tsub = pair * 2 + half
                        nc.tensor.matmul(
                            ps_pair[pair][:, half * SBW:(half + 1) * SBW],
                            lhsT=xap0(k)[:, tsub * 128:(tsub + 1) * 128],
                            rhs=wvap(k),
                            start=(k == 0), stop=False,
                            skip_group_check=True)
        for pair in range(2):
            for half in range(2):
                tsub = pair * 2 + half
                for k in range(3 * KC4, KC):
                    nc.tensor.matmul(
                        ps_pair[pair][:, half * SBW:(half + 1) * SBW],
                        lhsT=xap0(k)[:, tsub * 128:(tsub + 1) * 128],
                        rhs=wvap(k),
                        start=False, stop=(k == KC - 1),
                        skip_group_check=True)
            for half in range(2):
                nc.vector.tensor_copy(
                    vt[pair * 2 + half],
                    ps_pair[pair][:, half * SBW:(half + 1) * SBW])

        for n in range(1, NSB):
            xp = xpanel(xvT, n, f"xpv{n}")
            for pair in range(2):
                ps = ps_big.tile([128, W2], f32, tag="mm", name="ps_v")
                for half in range(2):
                    tsub = pair * 2 + half
                    for k in range(KC):
                        nc.tensor.matmul(
                            ps[:, half * SBW:(half + 1) * SBW],
                            lhsT=xap(xp, k)[:, tsub * 128:(tsub + 1) * 128],
                            rhs=wvap(k),
                            start=(k == 0), stop=(k == KC - 1))
                for half in range(2):
                    t = n * 4 + pair * 2 + half
                    nc.vector.tensor_copy(
                        vt[t], ps[:, half * SBW:(half + 1) * SBW])

        # ---- K projection (channels-major, all heads, 512-col blocks) --
        wk_sb = wpair(wkT, "wk")

        def proj_mms(w_sb, xp, m, pool_tag="mm"):
            pool = ps_q if pool_tag == "q" else ps_big
            ps = pool.tile([128, SBW], f32, tag=pool_tag, name="ps_proj")
            for k in range(KC):
                nc.tensor.matmul(
                    ps, lhsT=wap(w_sb, k, m),
                    rhs=xap(xp, k), start=(k == 0), stop=(k == KC - 1))
            return ps

        def proj_block(w_sb, b_sb, xp, m, out_ap):
            # out_ap [128, 512] = W_m x(+bias) for one 512-col s-block
            ps = proj_mms(w_sb, xp, m)
            nc.vector.tensor_scalar_add(out_ap, ps, b_sb[:, m:m + 1])

        def filler_qblockA(m, qt_next, xq_next):
            # filler Q-proj (first s-block): spare-bank psum; bias on DVE
            # so the ACT stream stays clear for attention exps
            psA = proj_mms(wq_sb, xq_next[0], m, pool_tag="q")
            nc.vector.tensor_scalar_add(
                qt_next[m][:, 0:SBW], psA, bq_sb[:, m:m + 1])

        def filler_qblockB(m, qt_next, xq_next):
            # second s-block, scheduled mid-head where the PE would
            # otherwise be exp-paced
            psB = proj_mms(wq_sb, xq_next[1], m)
            nc.vector.tensor_scalar_add(
                qt_next[m][:, SBW:W2], psB, bq_sb[:, m:m + 1])

        for b in range(NSB):
            xp = xpanel(xkT, b, f"xpk{b}")
            for m in range(NH):
                proj_block(wk_sb, bk_sb, xp, m,
                           kt[m][:, b * SBW:(b + 1) * SBW])

        # Q panels for s-half 0, then wq/wo (ring-slot gated behind wv/wk)
        xq_panels = [xpanel(xqT, 0, "xpq0"), xpanel(xqT, 1, "xpq1")]
        # wq/wo enqueues stall on the weight-pool ring until V/K proj
        # consume wv/wk; they must not sit in the scalar (ACT) stream or
        # they would fence the attention exps behind that wait.
        wq_sb = wpair(wqT, "wq")
        wo_sb = []
        for i in range(2):
            w = wp.tile([128, NH // 2, d], bf16, tag="w", name=f"wo{i}")
            nc.sync.dma_start(
                w, woT[:, i * (NH // 2):(i + 1) * (NH // 2)])
            wo_sb.append(w)

        qt_cur = [qtp.tile([128, W2], bf16, tag=f"qt{h}", name=f"qt{h}_0")
                  for h in range(NH)]
        # m == NH-1 is deferred: its blocks are drip-fed through head 0's
        # attention iterations as PE filler (it isn't read until head 3)
        for b in range(2):
            for m in range(NH - 1):
                proj_block(wq_sb, bq_sb, xq_panels[b], m,
                           qt_cur[m][:, b * SBW:(b + 1) * SBW])

        ot_prev = None
        qt_next = None
        xq_next = None
        lh_state = {}

        def outproj_half(dd, sp_, ot_tiles, half, zeng=None):
            # half-width out-proj block from the spare 1-bank psum (ps_q):
            # keeps the ps_big ring exclusively rotating scores<->exp, so
            # score matmuls never serialize behind outproj drains
            ps = ps_q.tile([128, SBW], f32, tag="q",
                           name=f"ps_zh{dd}_{half}")
            for eb in range(NH):
                nc.tensor.matmul(
                    ps,
                    lhsT=wo_sb[eb // 2][:, eb % 2, dd * 128:(dd + 1) * 128],
                    rhs=ot_tiles[eb][:, half * SBW:(half + 1) * SBW],
                    start=(eb == 0), stop=(eb == NH - 1),
                    skip_group_check=True)
            zt = zpool.tile([128, SBW], bf16, tag="z", name=f"zh{dd}_{half}")
            nc.vector.tensor_copy(zt, ps)
            (zeng or nc.sync).dma_start(
                zT[dd * 128:(dd + 1) * 128,
                   sp_ * W2 + half * SBW:sp_ * W2 + (half + 1) * SBW], zt)

        def outproj_dd(dd, sp, ot_tiles, cast_dve=False, zeng=None):
            # z[dd-block, s-half sp] accumulated over all 4 local heads
            ps = ps_big.tile([128, W2], f32, tag="mm", name=f"ps_z{dd}")
            for half in range(2):
                for eb in range(NH):
                    wo_ap = wo_sb[eb // 2][:, eb % 2,
                                           dd * 128:(dd + 1) * 128]
                    nc.tensor.matmul(
                        ps[:, half * SBW:(half + 1) * SBW],
                        lhsT=wo_ap,
                        rhs=ot_tiles[eb][:, half * SBW:(half + 1) * SBW],
                        start=(eb == 0), stop=(eb == NH - 1),
                        skip_group_check=True)
            zt = zpool.tile([128, W2], bf16, tag="z", name=f"z{dd}")
            if cast_dve:
                nc.vector.tensor_copy(zt, ps)
            else:
                nc.scalar.activation(
                    zt, ps, mybir.ActivationFunctionType.Copy)
            (zeng or nc.sync).dma_start(
                zT[dd * 128:(dd + 1) * 128, sp * W2:(sp + 1) * W2], zt)

        # ---- attention per (s-half, head) with PE filler blocks --------
        for sp in range(NSP):
            ot = [otp.tile([128, W2], bf16, tag=f"ot{h}", name=f"ot{h}_{sp}")
                  for h in range(NH)]
            if sp == 0:
                # prefetch Q panels + allocate qt for s-half 1; its proj
                # blocks are interleaved into this half's attention heads
                xq_next = [xpanel(xqT, 2, "xpq2"), xpanel(xqT, 3, "xpq3")]
                qt_next = [qtp.tile([128, W2], bf16, tag=f"qt{h}",
                                    name=f"qt{h}_1") for h in range(NH)]

            for h in range(NH):
                def flush_norm():
                    fn = lh_state.pop("deferred", None)
                    if fn:
                        fn()

                # ---- spread-extra queue: ~2 PE matmuls per iteration ----
                # ACT's exp throughput (1086ns/tile) slightly exceeds the
                # bare sc+pv PE cost (852ns/iter), and the 2-deep scores
                # psum ring caps ACT's lead at 2 tiles, so any mid-head
                # lump of PE filler forces ACT idle it can never repay.
                # Uniformly drip-feeding extras keeps every iteration
                # PE-bound with zero exp stalls.
                extras = []
                fill_st = {}

                def filler_chunk(m, blk, k0, xq_src, qt_tgt):
                    def go(m=m, blk=blk, k0=k0, xq_src=xq_src,
                           qt_tgt=qt_tgt):
                        if "ps" not in fill_st:
                            fill_st["ps"] = ps_q.tile(
                                [128, SBW], f32, tag="q",
                                name=f"fq{m}_{blk}")
                        ps = fill_st["ps"]
                        for k in (k0, k0 + 1):
                            nc.tensor.matmul(
                                ps, lhsT=wap(wq_sb, k, m),
                                rhs=xap(xq_src[blk], k),
                                start=(k == 0), stop=(k == KC - 1))
                        if k0 + 2 == KC:
                            nc.vector.tensor_scalar_add(
                                qt_tgt[m][:, blk * SBW:(blk + 1) * SBW],
                                fill_st.pop("ps"), bq_sb[:, m:m + 1])
                    return go

                op_st = {}

                def op_part(dd, half, part, ot_tiles):
                    def go(dd=dd, half=half, part=part, ot_tiles=ot_tiles):
                        if part == 0:
                            op_st[(dd, half)] = ps_q.tile(
                                [128, SBW], f32, tag="q",
                                name=f"ps_zh{dd}_{half}")
                        ps = (op_st[(dd, half)] if part == 0
                              else op_st.pop((dd, half)))
                        for eb in (part * 2, part * 2 + 1):
                            nc.tensor.matmul(
                                ps,
                                lhsT=wo_sb[eb // 2][:, eb % 2,
                                                    dd * 128:(dd + 1) * 128],
                                rhs=ot_tiles[eb][:,
                                                 half * SBW:(half + 1) * SBW],
                                start=(eb == 0), stop=(eb == NH - 1),
                                skip_group_check=True)
                        if part == 1:
                            # projects the PREVIOUS s-half's ot -> z cols
                            # [0, W2)
                            zt = zpool.tile([128, SBW], bf16, tag="z",
                                            name=f"zh{dd}_{half}")
                            nc.vector.tensor_copy(zt, ps)
                            nc.sync.dma_start(
                                zT[dd * 128:(dd + 1) * 128,
                                   half * SBW:(half + 1) * SBW], zt)
                    return go

                if sp == 0:
                    if h == 0:
                        for blk in range(2):
                            for k0 in range(0, KC, 2):
                                extras.append(filler_chunk(
                                    NH - 1, blk, k0, xq_panels, qt_cur))
                    else:
                        for blk in range(2):
                            for k0 in range(0, KC, 2):
                                extras.append(filler_chunk(
                                    h - 1, blk, k0, xq_next, qt_next))
                else:
                    for dd in (4 * h, 4 * h + 1, 4 * h + 2, 4 * h + 3):
                        for half in range(2):
                            extras.append(op_part(dd, half, 0, ot_prev))
                            extras.append(op_part(dd, half, 1, ot_prev))
                    if h == 0:
                        # lump is free here: ACT has nothing pending
                        # before this head's first scores exist
                        filler_qblockA(3, qt_next, xq_next)
                        filler_qblockB(3, qt_next, xq_next)
                        # previous s-half's last ot is read by the outproj
                        # parts - its deferred muls must come first
                        flush_norm()

                ei = [0]

                def pop_extra(n=1):
                    while n > 0 and ei[0] < len(extras):
                        extras[ei[0]]()
                        ei[0] += 1
                        n -= 1

                ops = ps_ops.tile([128, W2], f32, tag="ops", name="ps_pv")
                lps = ps_l.tile([33, SBW], f32, tag="l", name="ps_l")
                p2 = [None] * NTB
                pd = [None] * (NTB // 2)
                qd = [None] * (NTB // 4)
                od = [None] * 3
                last_head = (sp == NSP - 1 and h == NH - 1)

                def sc_exp(i, h=h, p2=p2):
                    ps = ps_big.tile([128, W2], f32, tag="mm",
                                     name=f"ps_sc{i}")
                    kb = kt[h][:, i * 128:(i + 1) * 128]
                    qth = qt_cur[h]
                    for half in range(2):
                        nc.tensor.matmul(
                            ps[:, half * SBW:(half + 1) * SBW],
                            lhsT=kb,
                            rhs=qth[:, half * SBW:(half + 1) * SBW],
                            start=True, stop=True)
                    p2[i] = p2p.tile([128, W2], bf16, tag="p",
                                     name=f"p{h}_{i}")
                    nc.scalar.activation(p2[i], ps, Exp, scale=SCALE)

                def pv(i, h=h, ops=ops, p2=p2, pd=pd, qd=qd, od=od,
                       last_head=last_head):
                    vb = vt[i][:, h * 128:(h + 1) * 128]
                    for half in range(2):
                        nc.tensor.matmul(
                            ops[:, half * SBW:(half + 1) * SBW],
                            lhsT=vb,
                            rhs=p2[i][:, half * SBW:(half + 1) * SBW],
                            start=(i == 0), stop=(i == NTB - 1),
                            skip_group_check=True)
                    if i == NTB - 1 and last_head:
                        return  # last head sums tiles 14/15 straight off p2
                    if i % 2 == 1:
                        j = i // 2
                        pd[j] = pap.tile([128, W2], bf16, tag="pd",
                                         name=f"pd{j}")
                        nc.vector.tensor_add(pd[j], p2[i - 1], p2[i])
                    if i % 4 == 3:
                        m2 = i // 4
                        qd[m2] = pap.tile([128, W2], bf16, tag="qd",
                                          name=f"qd{m2}")
                        nc.vector.tensor_add(qd[m2], pd[2 * m2],
                                             pd[2 * m2 + 1])
                    if i == 7:
                        od[0] = pap.tile([128, W2], bf16, tag="od",
                                         name="od01")
                        nc.vector.tensor_add(od[0], qd[0], qd[1])
                    if i == 15 and not last_head:
                        od[1] = pap.tile([128, W2], bf16, tag="od",
                                         name="od23")
                        nc.vector.tensor_add(od[1], qd[2], qd[3])
                        od[2] = pap.tile([128, W2], bf16, tag="sd",
                                         name="sd")
                        nc.vector.tensor_add(od[2], od[0], od[1])

                def lsum_g(src, first, last, lps=lps):
                    # one l-accumulation group: src summed over partitions
                    for half in range(2):
                        nc.tensor.matmul(
                            lps[32 * half:32 * half + 1, :],
                            lhsT=ones_col,
                            rhs=src[:, half * SBW:(half + 1) * SBW],
                            start=first, stop=last,
                            skip_group_check=True)

                sc_exp(0)
                sc_exp(1)
                if sp == 1 and h > 0:
                    # small early cover for the exp(0) latency
                    pop_extra(2)
                for i in range(NTB):
                    # extras go FIRST within the iteration so the scores-
                    # ring WAR gate (sc(i+2) waits exp(i)) is already met
                    if sp == 0:
                        pop_extra(2 if i == 0 else 1)
                    elif h == 0 or i <= 13:
                        pop_extra(1)
                    if i == 4:
                        flush_norm()
                    if i + 2 < NTB:
                        sc_exp(i + 2)
                    if i == NTB - 1 and last_head:
                        # l finishes off exp tiles, ahead of the last PV,
                        # so the normalize chain overlaps the outproj tail
                        lsum_g(pd[6], False, False)
                        lsum_g(p2[NTB - 2], False, False)
                        lsum_g(p2[NTB - 1], False, True)
                    pv(i)
                    if last_head:
                        if i == 9:
                            lsum_g(od[0], True, False)
                        if i == 13:
                            lsum_g(qd[2], False, last=False)
                pop_extra(16)  # safety: nothing should remain
                if not last_head:
                    # single l-accumulation group off the full bf16 add
                    # tree; its deadline is the deferred normalize at the
                    # next head's i==4, so the late emission is free
                    lsum_g(od[2], True, True)

                if last_head:
                    # normalize happens in the tail (PE-broadcast variant)
                    lh_state["lps"] = lps
                    lh_state["ops"] = ops
                    continue
                # drain O~ off PSUM fast (frees accumulator for next head);
                # the ENTIRE normalize chain is deferred into the next
                # head's body (i==4) - ot[h] isn't read until the next
                # s-half / tail, and deferring keeps the boundary DVE
                # burst from delaying the op-part casts that rotate ps_q
                o_raw = smalls.tile([128, W2], f32, tag="o_raw",
                                    name=f"o_raw{h}")
                nc.vector.tensor_copy(o_raw, ops)

                def norm_all(h=h, o_raw=o_raw, lps=lps, ot=ot):
                    l_sb = smalls.tile([1, W2], f32, tag="l_sb",
                                       name="l_sb")
                    r_sb = smalls.tile([1, W2], f32, tag="r_sb",
                                       name="r_sb")
                    rb = smalls.tile([128, W2], f32, tag="rb", name="rb")
                    for half in range(2):
                        hs = slice(half * SBW, (half + 1) * SBW)
                        nc.vector.tensor_copy(
                            l_sb[:, hs], lps[32 * half:32 * half + 1, :])
                        nc.vector.reciprocal_approx_fast(r_sb[:, hs],
                                                         l_sb[:, hs])
                    for half in range(2):
                        hs = slice(half * SBW, (half + 1) * SBW)
                        nc.gpsimd.partition_broadcast(rb[:, hs],
                                                      r_sb[:, hs])
                        nc.vector.tensor_mul(ot[h][:, hs], o_raw[:, hs],
                                             rb[:, hs])
                lh_state["deferred"] = norm_all

            if sp == 0:
                ot_prev = ot
                qt_cur = qt_next
            else:
                # tail: out-projection for the last s-half. First two
                # blocks defer their eb=3 matmuls so the PE has work while
                # the last head's normalization finishes; final blocks
                # drain at half width so the cast/DMA tail is shorter.
                first_ps = []
                for dd in range(2):
                    ps = ps_big.tile([128, W2], f32, tag="mm",
                                     name=f"ps_z{dd}")
                    first_ps.append(ps)
                    for half in range(2):
                        for eb in range(NH - 1):
                            nc.tensor.matmul(
                                ps[:, half * SBW:(half + 1) * SBW],
                                lhsT=wo_sb[eb // 2][:, eb % 2,
                                                    dd * 128:(dd + 1) * 128],
                                rhs=ot[eb][:, half * SBW:(half + 1) * SBW],
                                start=(eb == 0), stop=False,
                                skip_group_check=True)
                # last head's normalize: 1/l broadcast across partitions
                # via a 1-row PE matmul (213ns) instead of the 1us gpsimd
                # broadcast, multiplied straight off the PV accumulator
                lps_l, ops_l = lh_state["lps"], lh_state["ops"]
                l_sb = smalls.tile([1, W2], f32, tag="l_sb", name="l_sb_lh")
                r_sb = smalls.tile([1, W2], f32, tag="r_sb", name="r_sb_lh")
                for half in range(2):
                    hs = slice(half * SBW, (half + 1) * SBW)
                    nc.vector.tensor_copy(
                        l_sb[:, hs], lps_l[32 * half:32 * half + 1, :])
                    nc.vector.reciprocal_approx_fast(r_sb[:, hs],
                                                     l_sb[:, hs])
                for half in range(2):
                    hs = slice(half * SBW, (half + 1) * SBW)
                    rbp = ps_q.tile([128, SBW], f32, tag="q",
                                    name=f"rb_ps{half}")
                    nc.tensor.matmul(rbp, lhsT=ones_row_f32,
                                     rhs=r_sb[:, hs],
                                     start=True, stop=True,
                                     skip_group_check=True)
                    rb_s = smalls.tile([128, SBW], f32, tag="rb_lh",
                                       name=f"rb_lh{half}")
                    nc.vector.tensor_copy(rb_s, rbp)
                    nc.vector.tensor_mul(ot[NH - 1][:, hs], ops_l[:, hs],
                                         rb_s)
                for dd in range(2):
                    ps = first_ps[dd]
                    for half in range(2):
                        nc.tensor.matmul(
                            ps[:, half * SBW:(half + 1) * SBW],
                            lhsT=wo_sb[1][:, 1, dd * 128:(dd + 1) * 128],
                            rhs=ot[3][:, half * SBW:(half + 1) * SBW],
                            start=False, stop=True, skip_group_check=True)
                    zt = zpool.tile([128, W2], bf16, tag="z", name=f"z{dd}")
                    if dd % 2 == 0:
                        nc.scalar.activation(
                            zt, ps, mybir.ActivationFunctionType.Copy)
                    else:
                        nc.vector.tensor_copy(zt, ps)
                    (nc.sync, nc.scalar)[dd % 2].dma_start(
                        zT[dd * 128:(dd + 1) * 128, W2:2 * W2], zt)
                for dd in range(2, KC - 2):
                    outproj_dd(dd, 1, ot, cast_dve=(dd % 2 == 1),
                               zeng=(nc.sync, nc.scalar)[dd % 2])
                for dd in range(KC - 2, KC):
                    for half in range(2):
                        hs = slice(half * SBW, (half + 1) * SBW)
                        ps = ps_big.tile([128, SBW], f32, tag="mm",
                                         name=f"ps_zf{dd}_{half}")
                        for eb in range(NH):
                            nc.tensor.matmul(
                                ps,
                                lhsT=wo_sb[eb // 2][:, eb % 2,
                                                    dd * 128:(dd + 1) * 128],
                                rhs=ot[eb][:, hs],
                                start=(eb == 0), stop=(eb == NH - 1),
                                skip_group_check=True)
                        zt = zpool.tile([128, SBW], bf16, tag="z",
                                        name=f"zf{dd}_{half}")
                        if half == 0:
                            nc.scalar.activation(
                                zt, ps, mybir.ActivationFunctionType.Copy)
                        else:
                            nc.vector.tensor_copy(zt, ps)
                        (nc.sync, nc.scalar)[half].dma_start(
                            zT[dd * 128:(dd + 1) * 128,
                               W2 + half * SBW:W2 + (half + 1) * SBW], zt)

    nc.compile()
    return nc


def _bf16(a):
    return np.ascontiguousarray(a).astype(ml_dtypes.bfloat16)


def _in_maps(inputs):
    q = np.asarray(inputs["query"], dtype=np.float32)
    k = np.asarray(inputs["key_in"], dtype=np.float32)
    v = np.asarray(inputs["value"], dtype=np.float32)
    Wq = np.asarray(inputs["Wq"], dtype=np.float32)
    Wk = np.asarray(inputs["Wk"], dtype=np.float32)
    Wv = np.asarray(inputs["Wv"], dtype=np.float32)
    Wo = np.asarray(inputs["Wo"], dtype=np.float32)
    bq = np.asarray(inputs["bq"], dtype=np.float32)
    bk = np.asarray(inputs["bk"], dtype=np.float32)

    def xlin(xb):
        # [n*128+p, k, t] = x[n*512+t, k*128+p] - one 8KB line per
        # (partition, k-half) DMA descriptor
        v4 = _bf16(xb).reshape(S // 512, 512, D // 128, 128)
        return np.ascontiguousarray(v4.transpose(0, 3, 2, 1)).reshape(
            S // 512 * 128, D // 128, 512)

    def wlin(Wsl):
        # [p, k, m] = W[m, k*128+p]
        v = _bf16(Wsl).reshape(CL, D // 128, 128)
        return np.ascontiguousarray(v.transpose(2, 1, 0))

    xT = [[xlin(x[b]) for b in range(B)] for x in (q, k, v)]
    maps = []
    for c in range(NCORES):
        b, g = divmod(c, TP)
        sl = slice(g * CL, (g + 1) * CL)
        wo_l = _bf16(Wo[:, sl]).T.reshape(CL // 128, 128, D)
        maps.append({
            "xqT": xT[0][b], "xkT": xT[1][b], "xvT": xT[2][b],
            "wqT": wlin(Wq[sl, :]), "wkT": wlin(Wk[sl, :]),
            "wvT": wlin(Wv[sl, :]),
            "woT": np.ascontiguousarray(wo_l.transpose(1, 0, 2)),
            "bq": np.ascontiguousarray(bq[sl]),
            "bk": np.ascontiguousarray(bk[sl]),
        })
    return maps


TRACE = False
TMPDIR = None
LAST_RESULT = None


def kernel(**inputs):
    global _NC, LAST_RESULT
    from concourse.bass_utils import run_bass_kernel_spmd

    if _NC is None:
        _NC = _build_nc()
    maps = _in_maps(inputs)
    res = run_bass_kernel_spmd(_NC, maps, core_ids=list(range(NCORES)),
                               trace=TRACE, tmpdir=TMPDIR)
    LAST_RESULT = res

    Wo = np.asarray(inputs["Wo"], dtype=np.float32)
    bv = np.asarray(inputs["bv"], dtype=np.float32)
    bo = np.asarray(inputs["bo"], dtype=np.float32)
    out = np.zeros((B, S, D), dtype=np.float32)
    for c in range(NCORES):
        b, _ = divmod(c, TP)
        out[b] += res.results[c]["zT"].astype(np.float32).T
    out += (Wo @ bv + bo)[None, None, :]
    return out


if __name__ == "__main__":
    _build_nc()
    print("build OK")



# revision 48
# speedup vs baseline: 1.1890x; 1.1890x over previous
"""Trainium2 Bass kernel for CustomMultiHeadAttention.

Problem: B=2, S=2048, D=2048, H=16 heads of Hd=128; y = MHA(q, k, v) with
torch-Linear-style projections (x @ W.T + b) and out projection.

Sharding (8 NeuronCores): data-parallel over batch (2 groups) x tensor-
parallel over heads (4 heads = 512 channels per core). Each core computes
its 4 heads' Q/K/V projections, attention, and a column-sharded partial of
the out projection; the host sums the 4 partials per batch and adds
bo + Wo.T @ bv (the V bias contribution commutes through attention's
convex combination, so it is folded into a host-side constant).

Per-core device program (all matmuls bf16, K=128 contraction):
  - V projected sequence-major first (only needs xv + Wv), then K
    channels-major for all 4 heads, then per 1024-column s-half: Q proj,
    software-pipelined attention, and the partial out-projection.
  - attention inner loop is issue-order pipelined: scores for key-block
    i+2 are issued before P~V of key-block i, so the PE never waits on
    the ACT exp.
  - softmax denominator: DVE sums exp tiles into quad partials (bf16
    pair + quad adds), then a ones-column matmul per quad accumulates l
    in PSUM - a quarter of the PE stream cost of per-tile ones matmuls.
  - PE idle gaps are filled by interleaving: Q projection blocks for the
    next s-half ride inside attention heads of the current s-half, and
    out-projection blocks of the previous s-half ride inside attention
    heads of the next one.
  - normalization uses reciprocal_approx_fast (18-bit) + gpsimd
    partition broadcast, multiplied into O~ straight from PSUM.
"""

import math

import numpy as np
import ml_dtypes

B = 2
S = 2048
D = 2048
HD = 128          # head dim
TP = 4            # head-group (tensor-parallel) factor
CL = D // TP      # 512 local channels = 4 heads per core
NCORES = 8

_NC = None


def _build_nc(s=S, d=D, cl=CL):
    """Build the per-core Bass program (SPMD: same program, 8 cores)."""
    from contextlib import ExitStack

    import concourse.bass as bass
    import concourse.mybir as mybir
    import concourse.tile as tile
    from concourse import bacc, bass_isa

    f32 = mybir.dt.float32
    bf16 = mybir.dt.bfloat16
    Exp = mybir.ActivationFunctionType.Exp

    SBW = 512                 # single matmul max free dim / PSUM bank width
    W2 = 2 * SBW              # paired two-bank tile width
    NSB = s // SBW            # 4 s-blocks of 512
    NSP = s // W2             # 2 s-halves of 1024
    NTB = s // 128            # 16 key/value seq blocks
    KC = d // 128             # 16 contraction chunks over model dim
    NH = cl // HD             # 4 local heads
    SCALE = 1.0 / math.sqrt(HD)

    nc = bacc.Bacc("TRN2", target_bir_lowering=False, debug=False)

    # Inputs are host-pre-shuffled to the exact SBUF tile layouts so every
    # DMA line is 8KB contiguous (1KB lines cap a queue at ~100GB/s).
    # x*: [n*128+p, k, t] = x[n*512+t, k*128+p];  w*: [p, k, m] = W[m, k*128+p]
    NSB_ = s // 512
    xqT = nc.dram_tensor("xqT", [NSB_ * 128, d // 128, 512], bf16,
                         kind="ExternalInput").ap()
    xkT = nc.dram_tensor("xkT", [NSB_ * 128, d // 128, 512], bf16,
                         kind="ExternalInput").ap()
    xvT = nc.dram_tensor("xvT", [NSB_ * 128, d // 128, 512], bf16,
                         kind="ExternalInput").ap()
    wqT = nc.dram_tensor("wqT", [128, d // 128, cl], bf16,
                         kind="ExternalInput").ap()
    wkT = nc.dram_tensor("wkT", [128, d // 128, cl], bf16,
                         kind="ExternalInput").ap()
    wvT = nc.dram_tensor("wvT", [128, d // 128, cl], bf16,
                         kind="ExternalInput").ap()
    woT = nc.dram_tensor("woT", [128, cl // 128, d], bf16,
                         kind="ExternalInput").ap()
    bq = nc.dram_tensor("bq", [cl], f32, kind="ExternalInput").ap()
    bk = nc.dram_tensor("bk", [cl], f32, kind="ExternalInput").ap()
    zT = nc.dram_tensor("zT", [d, s], bf16, kind="ExternalOutput").ap()

    with tile.TileContext(nc) as tc, ExitStack() as ctx:
        const = ctx.enter_context(tc.tile_pool(name="const", bufs=1))
        wp = ctx.enter_context(tc.tile_pool(name="weights", bufs=4))
        kvp = ctx.enter_context(tc.tile_pool(name="kv", bufs=1))
        panels = ctx.enter_context(tc.tile_pool(name="panels", bufs=6))
        qtp = ctx.enter_context(tc.tile_pool(name="qt", bufs=1))
        otp = ctx.enter_context(tc.tile_pool(name="ot", bufs=2))
        p2p = ctx.enter_context(tc.tile_pool(name="ptiles", bufs=7))
        pap = ctx.enter_context(tc.tile_pool(name="padd", bufs=2))
        smalls = ctx.enter_context(tc.tile_pool(name="small", bufs=1))
        zpool = ctx.enter_context(tc.tile_pool(name="zout", bufs=3))
        # PSUM: rotating pair-bank pool (4) + PV accum (2) + l rows (1)
        ps_big = ctx.enter_context(tc.tile_pool(name="ps_big", bufs=2,
                                                space="PSUM"))
        ps_ops = ctx.enter_context(tc.tile_pool(name="ps_ops", bufs=1,
                                                space="PSUM"))
        ps_l = ctx.enter_context(tc.tile_pool(name="ps_l", bufs=1,
                                              space="PSUM"))
        ps_q = ctx.enter_context(tc.tile_pool(name="ps_q", bufs=1,
                                              space="PSUM"))

        ones_col = const.tile([128, 1], bf16, tag="ones_col")
        nc.vector.memset(ones_col, 1.0)
        ones_row_f32 = const.tile([1, 128], f32, tag="ones_row")
        nc.vector.memset(ones_row_f32, 1.0)
        warm = const.tile([128, SBW], bf16, tag="warm")
        nc.vector.memset(warm, 0.0)
        ps_warm = ps_q.tile([128, SBW], f32, tag="q", name="ps_warm")
        for _ in range(15):
            nc.tensor.matmul(ps_warm[0:1, :], lhsT=ones_col, rhs=warm,
                             start=True, stop=True, skip_group_check=True)

        KC2 = KC // 2
        KC4 = KC // 4

        # Weights and panels are split into k-halves (separate tiles) so
        # consumers can start as soon as the first half lands. DMAs are
        # spread across the two HWDGE queues (sync + scalar).
        def wpair(w_dram, nm, engs=None):
            pair = []
            for i in range(2):
                w = wp.tile([128, KC2, cl], bf16, tag="w", name=f"{nm}{i}")
                (engs[i] if engs else nc.sync).dma_start(
                    w, w_dram[:, i * KC2:(i + 1) * KC2])
                pair.append(w)
            return pair

        def wap(pair, k, m):
            return pair[k // KC2][:, k % KC2, m * 128:(m + 1) * 128]

        kt = [kvp.tile([128, s], bf16, tag=f"kt{h}", name=f"kt{h}")
              for h in range(NH)]
        vt = [kvp.tile([128, cl], bf16, tag=f"vt{t}", name=f"vt{t}")
              for t in range(NTB)]

        def xpanel(x_dram, n, nm, engs=None):
            src_ = x_dram[n * 128:(n + 1) * 128]
            xps = []
            for i in range(2):
                xp = panels.tile([128, KC2, SBW], bf16, tag="xpanel",
                                 name=f"{nm}_{i}")
                (engs[i] if engs else nc.sync).dma_start(
                    xp, src_[:, i * KC2:(i + 1) * KC2])
                xps.append(xp)
            return xps

        def xap(pair, k):
            return pair[k // KC2][:, k % KC2, :]

        # V path first: its weight + first panel gate the first matmul.
        # Both are loaded as interleaved k-QUARTER tiles, enqueued in
        # consumption order (the 16 shared DMA engines process packets
        # roughly in enqueue order, so order == priority): the first V
        # matmuls need only 0.75MB in flight.
        wv_sb = []
        xp0 = []
        for i in range(4):
            wq_t = wp.tile([128, KC4, cl], bf16, tag="wv", name=f"wv{i}",
                           bufs=4)
            nc.sync.dma_start(wq_t, wvT[:, i * KC4:(i + 1) * KC4])
            wv_sb.append(wq_t)
            xq_t = panels.tile([128, KC4, SBW], bf16, tag="xpanel",
                               name=f"xpv0_{i}")
            nc.sync.dma_start(xq_t, xvT[0:128, i * KC4:(i + 1) * KC4])
            xp0.append(xq_t)

        def wvap(k):
            return wv_sb[k // KC4][:, k % KC4, :]

        def xap0(k):
            return xp0[k // KC4][:, k % KC4, :]

        def load_bias(b_dram, nm):
            # SWDGE: keeps the 512 tiny 4B packets off the HW DMA engines
            b_dma = const.tile([128, NH], f32, tag=f"{nm}d", name=f"{nm}d")
            nc.gpsimd.dma_start(b_dma,
                                b_dram.rearrange("(m p) -> p m", p=128))
            b_sb = const.tile([128, NH], f32, tag=nm, name=nm)
            nc.vector.tensor_copy(b_sb, b_dma)
            return b_sb

        bq_sb = load_bias(bq, "bq")
        bk_sb = load_bias(bk, "bk")

        # ---- V projection: vt[t][tt, e] = sum_d x[t*128+tt, d] Wv[e, d]
        # Panel 0 runs k-quarter-major (both psum pairs live) so its first
        # matmuls need only the first quarters of wv + xpv0. The last
        # quarter goes pair-major so pair-0 drains overlap pair-1 matmuls.
        ps_pair = [ps_big.tile([128, W2], f32, tag="mm", name=f"ps_v0{p}")
                   for p in range(2)]
        for kq in range(3):
            for k in range(kq * KC4, (kq + 1) * KC4):
                for pair in range(2):
                    for half in range(2):
                        tsub = pair * 2 + half
                        nc.tensor.matmul(
                            ps_pair[pair][:, half * SBW:(half + 1) * SBW],
                            lhsT=xap0(k)[:, tsub * 128:(tsub + 1) * 128],
                            rhs=wvap(k),
                            start=(k == 0), stop=False,
                            skip_group_check=True)
        for pair in range(2):
            for half in range(2):
                tsub = pair * 2 + half
                for k in range(3 * KC4, KC):
                    nc.tensor.matmul(
                        ps_pair[pair][:, half * SBW:(half + 1) * SBW],
                        lhsT=xap0(k)[:, tsub * 128:(tsub + 1) * 128],
                        rhs=wvap(k),
                        start=False, stop=(k == KC - 1),
                        skip_group_check=True)
            for half in range(2):
                nc.vector.tensor_copy(
                    vt[pair * 2 + half],
                    ps_pair[pair][:, half * SBW:(half + 1) * SBW])

        for n in range(1, NSB):
            xp = xpanel(xvT, n, f"xpv{n}")
            for pair in range(2):
                ps = ps_big.tile([128, W2], f32, tag="mm", name="ps_v")
                for half in range(2):
                    tsub = pair * 2 + half
                    for k in range(KC):
                        nc.tensor.matmul(
                            ps[:, half * SBW:(half + 1) * SBW],
                            lhsT=xap(xp, k)[:, tsub * 128:(tsub + 1) * 128],
                            rhs=wvap(k),
                            start=(k == 0), stop=(k == KC - 1))
                for half in range(2):
                    t = n * 4 + pair * 2 + half
                    nc.vector.tensor_copy(
                        vt[t], ps[:, half * SBW:(half + 1) * SBW])

        # ---- K projection (channels-major, all heads, 512-col blocks) --
        wk_sb = wpair(wkT, "wk")

        def proj_mms(w_sb, xp, m, pool_tag="mm"):
            pool = ps_q if pool_tag == "q" else ps_big
            ps = pool.tile([128, SBW], f32, tag=pool_tag, name="ps_proj")
            for k in range(KC):
                nc.tensor.matmul(
                    ps, lhsT=wap(w_sb, k, m),
                    rhs=xap(xp, k), start=(k == 0), stop=(k == KC - 1))
            return ps

        def proj_block(w_sb, b_sb, xp, m, out_ap):
            # out_ap [128, 512] = W_m x(+bias) for one 512-col s-block
            ps = proj_mms(w_sb, xp, m)
            nc.vector.tensor_scalar_add(out_ap, ps, b_sb[:, m:m + 1])

        def filler_qblockA(m, qt_next, xq_next):
            # filler Q-proj (first s-block): spare-bank psum; bias on DVE
            # so the ACT stream stays clear for attention exps
            psA = proj_mms(wq_sb, xq_next[0], m, pool_tag="q")
            nc.vector.tensor_scalar_add(
                qt_next[m][:, 0:SBW], psA, bq_sb[:, m:m + 1])

        def filler_qblockB(m, qt_next, xq_next):
            # second s-block, scheduled mid-head where the PE would
            # otherwise be exp-paced
            psB = proj_mms(wq_sb, xq_next[1], m)
            nc.vector.tensor_scalar_add(
                qt_next[m][:, SBW:W2], psB, bq_sb[:, m:m + 1])

        for b in range(NSB):
            xp = xpanel(xkT, b, f"xpk{b}")
            for m in range(NH):
                proj_block(wk_sb, bk_sb, xp, m,
                           kt[m][:, b * SBW:(b + 1) * SBW])

        # Q panels for s-half 0, then wq/wo (ring-slot gated behind wv/wk)
        xq_panels = [xpanel(xqT, 0, "xpq0"), xpanel(xqT, 1, "xpq1")]
        # wq/wo enqueues stall on the weight-pool ring until V/K proj
        # consume wv/wk; they must not sit in the scalar (ACT) stream or
        # they would fence the attention exps behind that wait.
        wq_sb = wpair(wqT, "wq")
        wo_sb = []
        for i in range(2):
            w = wp.tile([128, NH // 2, d], bf16, tag="w", name=f"wo{i}")
            nc.sync.dma_start(
                w, woT[:, i * (NH // 2):(i + 1) * (NH // 2)])
            wo_sb.append(w)

        qt_cur = [qtp.tile([128, W2], bf16, tag=f"qt{h}", name=f"qt{h}_0")
                  for h in range(NH)]
        # m == NH-1 is deferred: its blocks are drip-fed through head 0's
        # attention iterations as PE filler (it isn't read until head 3)
        for b in range(2):
            for m in range(NH - 1):
                proj_block(wq_sb, bq_sb, xq_panels[b], m,
                           qt_cur[m][:, b * SBW:(b + 1) * SBW])

        ot_prev = None
        qt_next = None
        xq_next = None
        lh_state = {}

        def outproj_half(dd, sp_, ot_tiles, half, zeng=None):
            # half-width out-proj block from the spare 1-bank psum (ps_q):
            # keeps the ps_big ring exclusively rotating scores<->exp, so
            # score matmuls never serialize behind outproj drains
            ps = ps_q.tile([128, SBW], f32, tag="q",
                           name=f"ps_zh{dd}_{half}")
            for eb in range(NH):
                nc.tensor.matmul(
                    ps,
                    lhsT=wo_sb[eb // 2][:, eb % 2, dd * 128:(dd + 1) * 128],
                    rhs=ot_tiles[eb][:, half * SBW:(half + 1) * SBW],
                    start=(eb == 0), stop=(eb == NH - 1),
                    skip_group_check=True)
            zt = zpool.tile([128, SBW], bf16, tag="z", name=f"zh{dd}_{half}")
            nc.vector.tensor_copy(zt, ps)
            (zeng or nc.sync).dma_start(
                zT[dd * 128:(dd + 1) * 128,
                   sp_ * W2 + half * SBW:sp_ * W2 + (half + 1) * SBW], zt)

        def outproj_dd(dd, sp, ot_tiles, cast_dve=False, zeng=None):
            # z[dd-block, s-half sp] accumulated over all 4 local heads
            ps = ps_big.tile([128, W2], f32, tag="mm", name=f"ps_z{dd}")
            for half in range(2):
                for eb in range(NH):
                    wo_ap = wo_sb[eb // 2][:, eb % 2,
                                           dd * 128:(dd + 1) * 128]
                    nc.tensor.matmul(
                        ps[:, half * SBW:(half + 1) * SBW],
                        lhsT=wo_ap,
                        rhs=ot_tiles[eb][:, half * SBW:(half + 1) * SBW],
                        start=(eb == 0), stop=(eb == NH - 1),
                        skip_group_check=True)
            zt = zpool.tile([128, W2], bf16, tag="z", name=f"z{dd}")
            if cast_dve:
                nc.vector.tensor_copy(zt, ps)
            else:
                nc.scalar.activation(
                    zt, ps, mybir.ActivationFunctionType.Copy)
            (zeng or nc.sync).dma_start(
                zT[dd * 128:(dd + 1) * 128, sp * W2:(sp + 1) * W2], zt)

        # ---- attention per (s-half, head) with PE filler blocks --------
        for sp in range(NSP):
            ot = [otp.tile([128, W2], bf16, tag=f"ot{h}", name=f"ot{h}_{sp}")
                  for h in range(NH)]
            if sp == 0:
                # prefetch Q panels + allocate qt for s-half 1; its proj
                # blocks are interleaved into this half's attention heads
                xq_next = [xpanel(xqT, 2, "xpq2"), xpanel(xqT, 3, "xpq3")]
                qt_next = [qtp.tile([128, W2], bf16, tag=f"qt{h}",
                                    name=f"qt{h}_1") for h in range(NH)]

            for h in range(NH):
                def flush_norm():
                    fn = lh_state.pop("deferred", None)
                    if fn:
                        fn()

                # ---- spread-extra queue: ~2 PE matmuls per iteration ----
                # ACT's exp throughput (1086ns/tile) slightly exceeds the
                # bare sc+pv PE cost (852ns/iter), and the 2-deep scores
                # psum ring caps ACT's lead at 2 tiles, so any mid-head
                # lump of PE filler forces ACT idle it can never repay.
                # Uniformly drip-feeding extras keeps every iteration
                # PE-bound with zero exp stalls.
                extras = []
                fill_st = {}

                def filler_chunk(m, blk, k0, xq_src, qt_tgt):
                    def go(m=m, blk=blk, k0=k0, xq_src=xq_src,
                           qt_tgt=qt_tgt):
                        if "ps" not in fill_st:
                            fill_st["ps"] = ps_q.tile(
                                [128, SBW], f32, tag="q",
                                name=f"fq{m}_{blk}")
                        ps = fill_st["ps"]
                        for k in (k0, k0 + 1):
                            nc.tensor.matmul(
                                ps, lhsT=wap(wq_sb, k, m),
                                rhs=xap(xq_src[blk], k),
                                start=(k == 0), stop=(k == KC - 1))
                        if k0 + 2 == KC:
                            nc.vector.tensor_scalar_add(
                                qt_tgt[m][:, blk * SBW:(blk + 1) * SBW],
                                fill_st.pop("ps"), bq_sb[:, m:m + 1])
                    return go

                op_st = {}

                def op_part(dd, half, part, ot_tiles):
                    def go(dd=dd, half=half, part=part, ot_tiles=ot_tiles):
                        if part == 0:
                            op_st[(dd, half)] = ps_q.tile(
                                [128, SBW], f32, tag="q",
                                name=f"ps_zh{dd}_{half}")
                        ps = (op_st[(dd, half)] if part == 0
                              else op_st.pop((dd, half)))
                        for eb in (part * 2, part * 2 + 1):
                            nc.tensor.matmul(
                                ps,
                                lhsT=wo_sb[eb // 2][:, eb % 2,
                                                    dd * 128:(dd + 1) * 128],
                                rhs=ot_tiles[eb][:,
                                                 half * SBW:(half + 1) * SBW],
                                start=(eb == 0), stop=(eb == NH - 1),
                                skip_group_check=True)
                        if part == 1:
                            # projects the PREVIOUS s-half's ot -> z cols
                            # [0, W2)
                            zt = zpool.tile([128, SBW], bf16, tag="z",
                                            name=f"zh{dd}_{half}")
                            nc.vector.tensor_copy(zt, ps)
                            nc.sync.dma_start(
                                zT[dd * 128:(dd + 1) * 128,
                                   half * SBW:(half + 1) * SBW], zt)
                    return go

                if sp == 0:
                    if h == 0:
                        for blk in range(2):
                            for k0 in range(0, KC, 2):
                                extras.append(filler_chunk(
                                    NH - 1, blk, k0, xq_panels, qt_cur))
                    else:
                        for blk in range(2):
                            for k0 in range(0, KC, 2):
                                extras.append(filler_chunk(
                                    h - 1, blk, k0, xq_next, qt_next))
                else:
                    for dd in (4 * h, 4 * h + 1, 4 * h + 2, 4 * h + 3):
                        for half in range(2):
                            extras.append(op_part(dd, half, 0, ot_prev))
                            extras.append(op_part(dd, half, 1, ot_prev))
                    if h == 0:
                        # lump is free here: ACT has nothing pending
                        # before this head's first scores exist
                        filler_qblockA(3, qt_next, xq_next)
                        filler_qblockB(3, qt_next, xq_next)
                        # previous s-half's last ot is read by the outproj
                        # parts - its deferred muls must come first
                        flush_norm()

                ei = [0]

                def pop_extra(n=1):
                    while n > 0 and ei[0] < len(extras):
                        extras[ei[0]]()
                        ei[0] += 1
                        n -= 1

                ops = ps_ops.tile([128, W2], f32, tag="ops", name="ps_pv")
                lps = ps_l.tile([33, SBW], f32, tag="l", name="ps_l")
                p2 = [None] * NTB
                pd = [None] * (NTB // 2)
                qd = [None] * (NTB // 4)
                od = [None] * 3
                last_head = (sp == NSP - 1 and h == NH - 1)

                def sc_exp(i, h=h, p2=p2):
                    ps = ps_big.tile([128, W2], f32, tag="mm",
                                     name=f"ps_sc{i}")
                    kb = kt[h][:, i * 128:(i + 1) * 128]
                    qth = qt_cur[h]
                    for half in range(2):
                        nc.tensor.matmul(
                            ps[:, half * SBW:(half + 1) * SBW],
                            lhsT=kb,
                            rhs=qth[:, half * SBW:(half + 1) * SBW],
                            start=True, stop=True)
                    p2[i] = p2p.tile([128, W2], bf16, tag="p",
                                     name=f"p{h}_{i}")
                    nc.scalar.activation(p2[i], ps, Exp, scale=SCALE)

                def pv(i, h=h, ops=ops, p2=p2, pd=pd, qd=qd, od=od,
                       last_head=last_head):
                    vb = vt[i][:, h * 128:(h + 1) * 128]
                    for half in range(2):
                        nc.tensor.matmul(
                            ops[:, half * SBW:(half + 1) * SBW],
                            lhsT=vb,
                            rhs=p2[i][:, half * SBW:(half + 1) * SBW],
                            start=(i == 0), stop=(i == NTB - 1),
                            skip_group_check=True)
                    if i == NTB - 1:
                        return  # final adds are emitted ahead of these mms
                    if i % 2 == 1:
                        j = i // 2
                        pd[j] = pap.tile([128, W2], bf16, tag="pd",
                                         name=f"pd{j}")
                        nc.vector.tensor_add(pd[j], p2[i - 1], p2[i])
                    if i % 4 == 3:
                        m2 = i // 4
                        qd[m2] = pap.tile([128, W2], bf16, tag="qd",
                                          name=f"qd{m2}")
                        nc.vector.tensor_add(qd[m2], pd[2 * m2],
                                             pd[2 * m2 + 1])
                    if i == 7:
                        od[0] = pap.tile([128, W2], bf16, tag="od",
                                         name="od01")
                        nc.vector.tensor_add(od[0], qd[0], qd[1])

                def lsum_g(src, first, last, lps=lps):
                    # one l-accumulation group: src summed over partitions
                    for half in range(2):
                        nc.tensor.matmul(
                            lps[32 * half:32 * half + 1, :],
                            lhsT=ones_col,
                            rhs=src[:, half * SBW:(half + 1) * SBW],
                            start=first, stop=last,
                            skip_group_check=True)

                sc_exp(0)
                sc_exp(1)
                if sp == 1 and h > 0:
                    # small early cover for the exp(0) latency
                    pop_extra(2)
                for i in range(NTB):
                    # extras go FIRST within the iteration so the scores-
                    # ring WAR gate (sc(i+2) waits exp(i)) is already met
                    if sp == 0:
                        pop_extra(2 if i == 0 else 1)
                    elif h == 0 or i <= 13:
                        pop_extra(1)
                    if i == 4:
                        flush_norm()
                    if i + 2 < NTB:
                        sc_exp(i + 2)
                    if i == NTB - 1:
                        if last_head:
                            # l finishes off exp tiles, ahead of the last
                            # PV, so the normalize chain overlaps the
                            # outproj tail
                            lsum_g(pd[6], False, False)
                            lsum_g(p2[NTB - 2], False, False)
                            lsum_g(p2[NTB - 1], False, True)
                        else:
                            # final pair/quad adds ahead of the PV mms:
                            # they only need exp(14)/exp(15), so the post-
                            # loop lsum group is never DVE-gated
                            pd[7] = pap.tile([128, W2], bf16, tag="pd",
                                             name="pd7")
                            nc.vector.tensor_add(pd[7], p2[NTB - 2],
                                                 p2[NTB - 1])
                            qd[3] = pap.tile([128, W2], bf16, tag="qd",
                                             name="qd3")
                            nc.vector.tensor_add(qd[3], pd[6], pd[7])
                    pv(i)
                    if i == 9:
                        lsum_g(od[0], True, False)
                    if i == 13:
                        lsum_g(qd[2], False, last=False)
                pop_extra(16)  # safety: nothing should remain
                if not last_head:
                    lsum_g(qd[3], False, True)

                if last_head:
                    # normalize happens in the tail (PE-broadcast variant)
                    lh_state["lps"] = lps
                    lh_state["ops"] = ops
                    continue
                # drain O~ off PSUM fast (frees accumulator for next head);
                # the ENTIRE normalize chain is deferred into the next
                # head's body (i==4) - ot[h] isn't read until the next
                # s-half / tail, and deferring keeps the boundary DVE
                # burst from delaying the op-part casts that rotate ps_q
                o_raw = smalls.tile([128, W2], f32, tag="o_raw",
                                    name=f"o_raw{h}")
                nc.vector.tensor_copy(o_raw, ops)

                def norm_all(h=h, o_raw=o_raw, lps=lps, ot=ot):
                    l_sb = smalls.tile([1, W2], f32, tag="l_sb",
                                       name="l_sb")
                    r_sb = smalls.tile([1, W2], f32, tag="r_sb",
                                       name="r_sb")
                    rb = smalls.tile([128, W2], f32, tag="rb", name="rb")
                    for half in range(2):
                        hs = slice(half * SBW, (half + 1) * SBW)
                        nc.vector.tensor_copy(
                            l_sb[:, hs], lps[32 * half:32 * half + 1, :])
                        nc.vector.reciprocal_approx_fast(r_sb[:, hs],
                                                         l_sb[:, hs])
                    for half in range(2):
                        hs = slice(half * SBW, (half + 1) * SBW)
                        nc.gpsimd.partition_broadcast(rb[:, hs],
                                                      r_sb[:, hs])
                        nc.vector.tensor_mul(ot[h][:, hs], o_raw[:, hs],
                                             rb[:, hs])
                lh_state["deferred"] = norm_all

            if sp == 0:
                ot_prev = ot
                qt_cur = qt_next
            else:
                # tail: out-projection for the last s-half. First two
                # blocks defer their eb=3 matmuls so the PE has work while
                # the last head's normalization finishes; final blocks
                # drain at half width so the cast/DMA tail is shorter.
                first_ps = []
                for dd in range(2):
                    ps = ps_big.tile([128, W2], f32, tag="mm",
                                     name=f"ps_z{dd}")
                    first_ps.append(ps)
                    for half in range(2):
                        for eb in range(NH - 1):
                            nc.tensor.matmul(
                                ps[:, half * SBW:(half + 1) * SBW],
                                lhsT=wo_sb[eb // 2][:, eb % 2,
                                                    dd * 128:(dd + 1) * 128],
                                rhs=ot[eb][:, half * SBW:(half + 1) * SBW],
                                start=(eb == 0), stop=False,
                                skip_group_check=True)
                # last head's normalize: 1/l broadcast across partitions
                # via a 1-row PE matmul (213ns) instead of the 1us gpsimd
                # broadcast, multiplied straight off the PV accumulator
                lps_l, ops_l = lh_state["lps"], lh_state["ops"]
                l_sb = smalls.tile([1, W2], f32, tag="l_sb", name="l_sb_lh")
                r_sb = smalls.tile([1, W2], f32, tag="r_sb", name="r_sb_lh")
                for half in range(2):
                    hs = slice(half * SBW, (half + 1) * SBW)
                    nc.vector.tensor_copy(
                        l_sb[:, hs], lps_l[32 * half:32 * half + 1, :])
                    nc.vector.reciprocal_approx_fast(r_sb[:, hs],
                                                     l_sb[:, hs])
                for half in range(2):
                    hs = slice(half * SBW, (half + 1) * SBW)
                    rbp = ps_q.tile([128, SBW], f32, tag="q",
                                    name=f"rb_ps{half}")
                    nc.tensor.matmul(rbp, lhsT=ones_row_f32,
                                     rhs=r_sb[:, hs],
                                     start=True, stop=True,
                                     skip_group_check=True)
                    rb_s = smalls.tile([128, SBW], f32, tag="rb_lh",
                                       name=f"rb_lh{half}")
                    nc.vector.tensor_copy(rb_s, rbp)
                    nc.vector.tensor_mul(ot[NH - 1][:, hs], ops_l[:, hs],
                                         rb_s)
                for dd in range(2):
                    ps = first_ps[dd]
                    for half in range(2):
                        nc.tensor.matmul(
                            ps[:, half * SBW:(half + 1) * SBW],
                            lhsT=wo_sb[1][:, 1, dd * 128:(dd + 1) * 128],
                            rhs=ot[3][:, half * SBW:(half + 1) * SBW],
                            start=False, stop=True, skip_group_check=True)
                    zt = zpool.tile([128, W2], bf16, tag="z", name=f"z{dd}")
                    if dd % 2 == 0:
                        nc.scalar.activation(
                            zt, ps, mybir.ActivationFunctionType.Copy)
                    else:
                        nc.vector.tensor_copy(zt, ps)
                    (nc.sync, nc.scalar)[dd % 2].dma_start(
                        zT[dd * 128:(dd + 1) * 128, W2:2 * W2], zt)
                for dd in range(2, KC - 2):
                    outproj_dd(dd, 1, ot, cast_dve=(dd % 2 == 1),
                               zeng=(nc.sync, nc.scalar)[dd % 2])
                for dd in range(KC - 2, KC):
                    for half in range(2):
                        hs = slice(half * SBW, (half + 1) * SBW)
                        ps = ps_big.tile([128, SBW], f32, tag="mm",
                                         name=f"ps_zf{dd}_{half}")
                        for eb in range(NH):
                            nc.tensor.matmul(
                                ps,
                                lhsT=wo_sb[eb // 2][:, eb % 2,
                                                    dd * 128:(dd + 1) * 128],
                                rhs=ot[eb][:, hs],
                                start=(eb == 0), stop=(eb == NH - 1),
                                skip_group_check=True)
                        zt = zpool.tile([128, SBW], bf16, tag="z",
                                        name=f"zf{dd}_{half}")
                        if half == 0:
                            nc.scalar.activation(
                                zt, ps, mybir.ActivationFunctionType.Copy)
                        else:
                            nc.vector.tensor_copy(zt, ps)
                        (nc.sync, nc.scalar)[half].dma_start(
                            zT[dd * 128:(dd + 1) * 128,
                               W2 + half * SBW:W2 + (half + 1) * SBW], zt)

    nc.compile()
    return nc


def _bf16(a):
    return np.ascontiguousarray(a).astype(ml_dtypes.bfloat16)


def _in_maps(inputs):
    q = np.asarray(inputs["query"], dtype=np.float32)
    k = np.asarray(inputs["key_in"], dtype=np.float32)
    v = np.asarray(inputs["value"], dtype=np.float32)
    Wq = np.asarray(inputs["Wq"], dtype=np.float32)
    Wk = np.asarray(inputs["Wk"], dtype=np.float32)
    Wv = np.asarray(inputs["Wv"], dtype=np.float32)
    Wo = np.asarray(inputs["Wo"], dtype=np.float32)
    bq = np.asarray(inputs["bq"], dtype=np.float32)
    bk = np.asarray(inputs["bk"], dtype=np.float32)

    def xlin(xb):
        # [n*128+p, k, t] = x[n*512+t, k*128+p] - one 8KB line per
        # (partition, k-half) DMA descriptor
        v4 = _bf16(xb).reshape(S // 512, 512, D // 128, 128)
        return np.ascontiguousarray(v4.transpose(0, 3, 2, 1)).reshape(
            S // 512 * 128, D // 128, 512)

    def wlin(Wsl):
        # [p, k, m] = W[m, k*128+p]
        v = _bf16(Wsl).reshape(CL, D // 128, 128)
        return np.ascontiguousarray(v.transpose(2, 1, 0))

    xT = [[xlin(x[b]) for b in range(B)] for x in (q, k, v)]
    maps = []
    for c in range(NCORES):
        b, g = divmod(c, TP)
        sl = slice(g * CL, (g + 1) * CL)
        wo_l = _bf16(Wo[:, sl]).T.reshape(CL // 128, 128, D)
        maps.append({
            "xqT": xT[0][b], "xkT": xT[1][b], "xvT": xT[2][b],
            "wqT": wlin(Wq[sl, :]), "wkT": wlin(Wk[sl, :]),
            "wvT": wlin(Wv[sl, :]),
            "woT": np.ascontiguousarray(wo_l.transpose(1, 0, 2)),
            "bq": np.ascontiguousarray(bq[sl]),
            "bk": np.ascontiguousarray(bk[sl]),
        })
    return maps


TRACE = False
TMPDIR = None
LAST_RESULT = None


def kernel(**inputs):
    global _NC, LAST_RESULT
    from concourse.bass_utils import run_bass_kernel_spmd

    if _NC is None:
        _NC = _build_nc()
    maps = _in_maps(inputs)
    res = run_bass_kernel_spmd(_NC, maps, core_ids=list(range(NCORES)),
                               trace=TRACE, tmpdir=TMPDIR)
    LAST_RESULT = res

    Wo = np.asarray(inputs["Wo"], dtype=np.float32)
    bv = np.asarray(inputs["bv"], dtype=np.float32)
    bo = np.asarray(inputs["bo"], dtype=np.float32)
    out = np.zeros((B, S, D), dtype=np.float32)
    for c in range(NCORES):
        b, _ = divmod(c, TP)
        out[b] += res.results[c]["zT"].astype(np.float32).T
    out += (Wo @ bv + bo)[None, None, :]
    return out


if __name__ == "__main__":
    _build_nc()
    print("build OK")

